# revision 4
# baseline (speedup 1.0000x reference)
"""Trainium2 Bass kernel for nn_CAWeightedFusion.

Math note: in the reference, ra/ca are softmaxed over the flattened spatial
axis N=H*W and then immediately mean-pooled over that same axis. A softmax
row sums to exactly 1, so mean(ra) = mean(ca) = 1/N elementwise and the whole
QKV/attention pipeline cancels out of the output:

    g[b,c] = mean_hw(rgb[b,c]) + mean_hw(chm[b,c]) + 2/N
    out    = sigmoid(relu(g @ w_mlp1.T) @ w_mlp2.T)[:, :, None, None]

What remains is a memory-bound spatial reduction plus a tiny MLP. We go
batch-parallel: core b reduces batch b (rgb+chm, shipped bf16), fusing the
first MLP layer into the reduction as 64 PSUM-accumulated matmuls
(w1_chunk.T[128,24] @ x_chunk[128,512]), then one free-axis reduce, a
bias+relu (the 1/N scale and the 2/N constant folded into scale/bias), the
1x24 second layer, and a sigmoid.
"""

import numpy as np
import ml_dtypes

B, C, HW = 8, 512, 4096
NCORES = 8
HID = 24

_CACHE = {}


def _build_program():
    import concourse.bacc as bacc
    import concourse.bass as bass
    import concourse.mybir as mybir
    import concourse.tile as tile

    bf16 = mybir.dt.bfloat16
    f32 = mybir.dt.float32
    ts = bass.ts

    nc = bacc.Bacc(
        "TRN2",
        target_bir_lowering=False,
        debug=False,
        enable_asserts=False,
        num_devices=NCORES,
    )

    xr = nc.dram_tensor("xr", [C, HW], bf16, kind="ExternalInput")
    xc = nc.dram_tensor("xc", [C, HW], bf16, kind="ExternalInput")
    # wt[:, 24k:24k+24] = w_mlp1[:, 128k:128k+128].T  (k = 0..3)
    wt = nc.dram_tensor("wt", [128, 4 * HID], f32, kind="ExternalInput")
    b1 = nc.dram_tensor("b1", [HID, 1], f32, kind="ExternalInput")
    w2t = nc.dram_tensor("w2t", [HID, 1], f32, kind="ExternalInput")
    out = nc.dram_tensor("out", [1, 1], f32, kind="ExternalOutput")

    # Chunk schedule: (modality, row_chunk k, col_start, ncols). Full tiles
    # first, shrinking chunks last so the final reduce after the last DMA is
    # short. Each chunk's row-sum partial feeds one tiny PE matmul against
    # w1 chunk k, PSUM-accumulated into [24,1].
    chunks = []
    for m, k in [(0, 0), (0, 1), (0, 2), (0, 3), (1, 0), (1, 1)]:
        chunks.append((m, k, 0, HW))
    for j in range(2):
        chunks.append((1, 2, j * 2048, 2048))
    for j in range(4):
        chunks.append((1, 3, j * 1024, 1024))

    # Greedy DVE/ACT balance on a simple cost/arrival model (ns).
    bw = 0.33e3  # bytes per ns per-core HBM
    avail, acc_bytes = [], 0
    for (_, _, _, n) in chunks:
        acc_bytes += 128 * n * 2
        avail.append(acc_bytes / bw)
    eng_free = {"dve": 0.0, "act": 0.0}
    cost = {
        "dve": lambda n: (120 + n) / 0.96,
        "act": lambda n: (352 + n) / 1.2,
    }
    assign = []
    for i, (_, _, _, n) in enumerate(chunks):
        fin = {e: max(eng_free[e], avail[i]) + cost[e](n) for e in eng_free}
        e = min(fin, key=fin.get)
        eng_free[e] = fin[e]
        assign.append(e)
    n_dve = sum(1 for e in assign if e == "dve")
    n_act = len(assign) - n_dve

    with tile.TileContext(nc) as tc:
        with (
            tc.tile_pool(name="xp", bufs=5) as xp,
            tc.tile_pool(name="cst", bufs=1) as cst,
            tc.tile_pool(name="acc", bufs=1, space="PSUM") as accp,
            tc.tile_pool(name="eps", bufs=1, space="PSUM") as epsp,
            tc.tile_pool(name="sb", bufs=1) as sb,
        ):
            wt_t = cst.tile([128, 4 * HID], f32)
            nc.sync.dma_start(wt_t[:], wt[:])
            b1_t = cst.tile([HID, 1], f32)
            nc.sync.dma_start(b1_t[:], b1[:])
            w2_t = cst.tile([HID, 1], f32)
            nc.sync.dma_start(w2_t[:], w2t[:])

            pdve = cst.tile([128, max(n_dve, 1)], f32)
            pact = cst.tile([128, max(n_act, 1)], f32)

            acc24 = accp.tile([HID, 1], f32)
            idx = {"dve": 0, "act": 0}
            for i, ((m, k, c0, n), e) in enumerate(zip(chunks, assign)):
                src = xr if m == 0 else xc
                xt = xp.tile([128, n], bf16)
                nc.sync.dma_start(xt[:], src[ts(k, 128), c0:c0 + n])
                col = idx[e]
                idx[e] += 1
                if e == "dve":
                    part = pdve[:, col:col + 1]
                    nc.vector.reduce_sum(part, xt[:], axis=mybir.AxisListType.X)
                else:
                    part = pact[:, col:col + 1]
                    nc.scalar.activation(
                        xt[:], xt[:], mybir.ActivationFunctionType.Copy,
                        accum_out=part,
                    )
                nc.tensor.matmul(
                    acc24[:],
                    wt_t[:, ts(k, HID)],
                    part,
                    start=(i == 0),
                    stop=(i == len(chunks) - 1),
                )

            h1 = sb.tile([HID, 1], f32)
            nc.scalar.activation(
                h1[:], acc24[:], mybir.ActivationFunctionType.Relu,
                bias=b1_t[:], scale=1.0 / HW,
            )
            g2 = epsp.tile([1, 1], f32)
            nc.tensor.matmul(g2[:], h1[:], w2_t[:], start=True, stop=True)
            gate = sb.tile([1, 1], f32)
            nc.scalar.activation(gate[:], g2[:], mybir.ActivationFunctionType.Sigmoid)
            nc.sync.dma_start(out[:], gate[:])

    nc.compile()
    return nc


def kernel(rgb, chm, w_rgb_qkv, b_rgb_qkv, w_chm_qkv, b_chm_qkv, w_mlp1, w_mlp2):
    from concourse.bass_utils import run_bass_kernel_spmd

    if "nc" not in _CACHE:
        _CACHE["nc"] = _build_program()
    nc = _CACHE["nc"]

    bf16 = ml_dtypes.bfloat16
    w1 = np.asarray(w_mlp1, dtype=np.float32)          # [24, 512]
    wt = np.empty((128, 4 * HID), dtype=np.float32)
    for k in range(4):
        wt[:, k * HID:(k + 1) * HID] = w1[:, k * 128:(k + 1) * 128].T
    b1 = (2.0 / HW) * w1.sum(axis=1, dtype=np.float64)
    b1 = b1.astype(np.float32).reshape(HID, 1)
    w2t = np.asarray(w_mlp2, dtype=np.float32).reshape(HID, 1)

    rgb = np.asarray(rgb).reshape(B, C, HW)
    chm = np.asarray(chm).reshape(B, C, HW)
    in_maps = []
    for b in range(B):
        in_maps.append({
            "xr": rgb[b].astype(bf16),
            "xc": chm[b].astype(bf16),
            "wt": wt,
            "b1": b1,
            "w2t": w2t,
        })

    res = run_bass_kernel_spmd(nc, in_maps, core_ids=list(range(NCORES)))
    _CACHE["last_results"] = res

    gates = np.stack([res.results[b]["out"].reshape(()) for b in range(B)])
    return gates.reshape(B, 1, 1, 1).astype(np.float32)


# revision 5
# speedup vs baseline: 1.0832x; 1.0832x over previous
"""Trainium2 Bass kernel for nn_CAWeightedFusion.

Math note: in the reference, ra/ca are softmaxed over the flattened spatial
axis N=H*W and then immediately mean-pooled over that same axis. A softmax
row sums to exactly 1, so mean(ra) = mean(ca) = 1/N elementwise and the whole
QKV/attention pipeline cancels out of the output:

    g[b,c] = mean_hw(rgb[b,c]) + mean_hw(chm[b,c]) + 2/N
    out    = sigmoid(relu(g @ w_mlp1.T) @ w_mlp2.T)[:, :, None, None]

What remains is a memory-bound spatial reduction plus a tiny MLP. We go
batch-parallel: core b reduces batch b (rgb+chm, shipped bf16), fusing the
first MLP layer into the reduction as 64 PSUM-accumulated matmuls
(w1_chunk.T[128,24] @ x_chunk[128,512]), then one free-axis reduce, a
bias+relu (the 1/N scale and the 2/N constant folded into scale/bias), the
1x24 second layer, and a sigmoid.
"""

import numpy as np
import ml_dtypes

B, C, HW = 8, 512, 4096
NCORES = 8
HID = 24

_CACHE = {}


def _build_program():
    import concourse.bacc as bacc
    import concourse.bass as bass
    import concourse.mybir as mybir
    import concourse.tile as tile

    bf16 = mybir.dt.bfloat16
    f32 = mybir.dt.float32
    ts = bass.ts

    nc = bacc.Bacc(
        "TRN2",
        target_bir_lowering=False,
        debug=False,
        enable_asserts=False,
        num_devices=NCORES,
    )

    xr = nc.dram_tensor("xr", [C, HW], bf16, kind="ExternalInput")
    xc = nc.dram_tensor("xc", [C, HW], bf16, kind="ExternalInput")
    # wt[:, 24k:24k+24] = w_mlp1[:, 128k:128k+128].T  (k = 0..3)
    wt = nc.dram_tensor("wt", [128, 4 * HID], f32, kind="ExternalInput")
    b1 = nc.dram_tensor("b1", [HID, 1], f32, kind="ExternalInput")
    w2t = nc.dram_tensor("w2t", [HID, 1], f32, kind="ExternalInput")
    out = nc.dram_tensor("out", [1, 1], f32, kind="ExternalOutput")

    # Chunk schedule: (modality, row_chunk k, col_start, ncols). Full tiles
    # first, shrinking chunks last so the final reduce after the last DMA is
    # short. Each chunk's row-sum partial feeds one tiny PE matmul against
    # w1 chunk k, PSUM-accumulated into [24,1].
    chunks = []
    for m, k in [(0, 0), (0, 1), (0, 2), (0, 3), (1, 0), (1, 1)]:
        chunks.append((m, k, 0, HW))
    for j in range(2):
        chunks.append((1, 2, j * 2048, 2048))
    for j in range(4):
        chunks.append((1, 3, j * 1024, 1024))

    # Brute-force DVE/ACT split minimizing simulated makespan (costs in ns,
    # measured on HW: DVE reduce (120+n)/0.96; ACT copy (352+n)/1.2 plus a
    # 279ns accumulator read).
    bw = 0.358e3  # bytes/ns per-core HBM
    avail, acc_bytes = [], 0
    for (_, _, _, n) in chunks:
        acc_bytes += 128 * n * 2
        avail.append(acc_bytes / bw)
    c_dve = [(120 + n) / 0.96 for (_, _, _, n) in chunks]
    c_act = [(352 + n) / 1.2 + 279 for (_, _, _, n) in chunks]
    best, assign = None, None
    for mask in range(1 << len(chunks)):
        td = ta = 0.0
        for i in range(len(chunks)):
            if mask >> i & 1:
                td = max(td, avail[i]) + c_dve[i]
            else:
                ta = max(ta, avail[i]) + c_act[i]
        mk = max(td, ta)
        if best is None or mk < best:
            best, assign = mk, ["dve" if mask >> i & 1 else "act" for i in range(len(chunks))]
    n_dve = sum(1 for e in assign if e == "dve")
    n_act = len(assign) - n_dve

    with tile.TileContext(nc) as tc:
        with (
            tc.tile_pool(name="xp", bufs=len(chunks)) as xp,
            tc.tile_pool(name="cst", bufs=1) as cst,
            tc.tile_pool(name="acc", bufs=1, space="PSUM") as accp,
            tc.tile_pool(name="eps", bufs=1, space="PSUM") as epsp,
            tc.tile_pool(name="sb", bufs=1) as sb,
        ):
            # Dummy sigmoid first in ScalarE program order: walrus then loads
            # an act table set containing sigmoid (sigmoid_and_others, which
            # also holds copy+relu) once at kernel start, instead of switching
            # sets in the critical tail.
            dummy = sb.tile([1, 1], f32)
            nc.gpsimd.memset(dummy[:], 0.0)
            dummy2 = sb.tile([1, 1], f32)
            nc.scalar.activation(
                dummy2[:], dummy[:], mybir.ActivationFunctionType.Sigmoid
            )

            pdve = cst.tile([128, max(n_dve, 1)], f32)
            pact = cst.tile([128, max(n_act, 1)], f32)
            wt_t = cst.tile([128, 4 * HID], f32)
            b1_t = cst.tile([HID, 1], f32)
            w2_t = cst.tile([HID, 1], f32)

            acc24 = accp.tile([HID, 1], f32)
            idx = {"dve": 0, "act": 0}
            mms = []
            for i, ((m, k, c0, n), e) in enumerate(zip(chunks, assign)):
                src = xr if m == 0 else xc
                xt = xp.tile([128, n], bf16)
                nc.sync.dma_start(xt[:], src[ts(k, 128), c0:c0 + n])
                col = idx[e]
                idx[e] += 1
                if e == "dve":
                    part = pdve[:, col:col + 1]
                    nc.vector.reduce_sum(part, xt[:], axis=mybir.AxisListType.X)
                else:
                    part = pact[:, col:col + 1]
                    nc.scalar.activation(
                        xt[:], xt[:], mybir.ActivationFunctionType.Copy,
                        accum_out=part,
                    )
                mms.append((k, part))

            # Const DMAs after the x posts on the sync queue: only needed for
            # the PE accumulation + epilogue, never gate the stream head.
            nc.sync.dma_start(wt_t[:], wt[:])
            nc.sync.dma_start(b1_t[:], b1[:])
            nc.sync.dma_start(w2_t[:], w2t[:])

            for i, (k, part) in enumerate(mms):
                nc.tensor.matmul(
                    acc24[:],
                    wt_t[:, ts(k, HID)],
                    part,
                    start=(i == 0),
                    stop=(i == len(mms) - 1),
                )

            h1 = sb.tile([HID, 1], f32)
            nc.scalar.activation(
                h1[:], acc24[:], mybir.ActivationFunctionType.Relu,
                bias=b1_t[:], scale=1.0 / HW,
            )
            g2 = epsp.tile([1, 1], f32)
            nc.tensor.matmul(g2[:], h1[:], w2_t[:], start=True, stop=True)
            gate = sb.tile([1, 1], f32)
            nc.scalar.activation(gate[:], g2[:], mybir.ActivationFunctionType.Sigmoid)
            nc.sync.dma_start(out[:], gate[:])

    nc.compile()
    return nc


def kernel(rgb, chm, w_rgb_qkv, b_rgb_qkv, w_chm_qkv, b_chm_qkv, w_mlp1, w_mlp2):
    from concourse.bass_utils import run_bass_kernel_spmd

    if "nc" not in _CACHE:
        _CACHE["nc"] = _build_program()
    nc = _CACHE["nc"]

    bf16 = ml_dtypes.bfloat16
    w1 = np.asarray(w_mlp1, dtype=np.float32)          # [24, 512]
    wt = np.empty((128, 4 * HID), dtype=np.float32)
    for k in range(4):
        wt[:, k * HID:(k + 1) * HID] = w1[:, k * 128:(k + 1) * 128].T
    b1 = (2.0 / HW) * w1.sum(axis=1, dtype=np.float64)
    b1 = b1.astype(np.float32).reshape(HID, 1)
    w2t = np.asarray(w_mlp2, dtype=np.float32).reshape(HID, 1)

    rgb = np.asarray(rgb).reshape(B, C, HW)
    chm = np.asarray(chm).reshape(B, C, HW)
    in_maps = []
    for b in range(B):
        in_maps.append({
            "xr": rgb[b].astype(bf16),
            "xc": chm[b].astype(bf16),
            "wt": wt,
            "b1": b1,
            "w2t": w2t,
        })

    res = run_bass_kernel_spmd(nc, in_maps, core_ids=list(range(NCORES)))
    _CACHE["last_results"] = res

    gates = np.stack([res.results[b]["out"].reshape(()) for b in range(B)])
    return gates.reshape(B, 1, 1, 1).astype(np.float32)
